# revision 19
# baseline (speedup 1.0000x reference)
"""CPT attention (QKV+LoRA -> fake-quant KV -> causal attention -> proj+LoRA)
as a Bass/Tile kernel on 8 TRN2 NeuronCores.

Sharding: data parallel over batch (2) x tensor parallel over heads (16/4=4
per core), Megatron-style. Each core computes qkv for its 4 heads from the
full hidden_states[b], runs causal attention locally, and produces a partial
projection output [T, C]; the host sums the 4 tensor-parallel partials per
batch and adds b_proj.

Device-side design (v2):
- All matmul operands f16, fp32 PSUM accum. Host pre-transposes everything.
- The quant scale 1/kv_scale is folded into the K/V columns of W_attn on the
  host, so fake_quant is 2 DVE passes (clip; magic-round) when kv_zp==0
  (3 passes otherwise). K is kept integer-valued (scale folded into the exp
  scale), V integer-valued (scale folded into W_proj).
- Scores are computed transposed (S^T[k, q]) per head pair with explicit PE
  row-group tile positions; exp on the scalar engine; causal diagonal masks
  multiplied on gpsimd.
- PV uses column-tiled matmul pairs: V_h0 -> PSUM partitions 0-63 and
  V_h1 -> partitions 64-127 stream two moving halves of ex concurrently,
  producing attnT[ch, q] directly (no transposes).
- Softmax denominators come from ones-stationary matmuls into a [2, 512]
  PSUM tile; reciprocal on DVE, replicated across partitions on gpsimd, and
  multiplied into the PV PSUM->SBUF evacuation.
- The whole kernel is one software-pipelined stream over 8 stages
  (4 q-blocks x 2 head pairs): scores(s) interleave with PV+den(s-1) and a
  filler queue of qkv/V/proj chunks sized to keep the PE busy while the
  scalar engine exps.
"""

import numpy as np

import concourse.bass as bass
import concourse.bacc as bacc
import concourse.mybir as mybir
import concourse.tile as tile
from concourse.bass_utils import run_bass_kernel_spmd

AF = mybir.ActivationFunctionType
OP = mybir.AluOpType

B, T, C = 2, 2048, 1024
H, HD = 16, 64
R = 16
ALPHA_OVER_R = 2.0
QMAX = 255.0
MAGIC = 12582912.0  # 1.5 * 2**23: fp32 add/sub rounds to nearest-even integer
N_CORES = 8
HPC = 4  # heads per core
CH = HPC * HD  # 256 v/q/k channels per core
NT = T // 128  # 16 T-tiles
NC_ = C // 128  # 8 C-tiles
F16 = mybir.dt.float16
F32 = mybir.dt.float32


def _build_body(nc, tc, d, use_bias, use_lora_attn, use_lora_proj, zp_zero):
    DBG = "dbg_qkT0" in d
    SCHED = globals()["SCHED"]
    import contextlib

    ctx = contextlib.ExitStack()
    with ctx:
        persist = ctx.enter_context(tc.tile_pool(name="persist", bufs=1))
        fqp = ctx.enter_context(tc.tile_pool(name="fqp", bufs=4))
        exq = ctx.enter_context(tc.tile_pool(name="exq", bufs=20))
        outp = ctx.enter_context(tc.tile_pool(name="outp", bufs=3))
        rcpp = ctx.enter_context(tc.tile_pool(name="rcpp", bufs=4))
        psS = ctx.enter_context(
            tc.tile_pool(name="psS", bufs=2, space=bass.MemorySpace.PSUM)
        )
        psQ = ctx.enter_context(
            tc.tile_pool(name="psQ", bufs=2, space=bass.MemorySpace.PSUM)
        )
        psPV = ctx.enter_context(
            tc.tile_pool(name="psPV", bufs=1, space=bass.MemorySpace.PSUM)
        )
        psD = ctx.enter_context(
            tc.tile_pool(name="psD", bufs=1, space=bass.MemorySpace.PSUM)
        )

        # ---- constants ----
        consts = persist.tile([128, 4], F32, tag="consts", name="consts")
        nc.sync.dma_start(consts[:, :], d["consts"][:, :])
        zp_ap = consts[:, 1:2]
        es_ap = consts[:, 3:4]  # 0.125 * kv_scale (scores use integer-valued K)
        maskt = persist.tile([128, 128], F16, tag="maskt", name="maskt")
        nc.sync.dma_start(maskt[:, :], d["masks"][:, :])
        ones16 = persist.tile([128, 1], F16, tag="ones16", name="ones16")
        nc.gpsimd.memset(ones16[:, :], 1.0)
        # rank-1 broadcast masks: mka selects out partitions 0-63, mkb 64-127
        mka = persist.tile([1, 128], F16, tag="mka", name="mka")
        nc.gpsimd.memset(mka[:, :], 0.0)
        nc.gpsimd.memset(mka[0:1, 0:64], 1.0)
        mkb = persist.tile([1, 128], F16, tag="mkb", name="mkb")
        nc.gpsimd.memset(mkb[:, :], 0.0)
        nc.gpsimd.memset(mkb[0:1, 64:128], 1.0)
        # preload the Exp activation table while DMAs stream in
        scr = persist.tile([1, 8], F32, tag="scr", name="scr")
        nc.scalar.activation(scr[0:1, 0:1], consts[0:1, 1:2], AF.Exp)
        if use_bias:
            ones_row = persist.tile([1, 512], F16, tag="ones_row", name="ones_row")
            nc.gpsimd.memset(ones_row[:, :], 1.0)
            bqk_row = persist.tile([1, 2 * CH], F16, tag="bqk_row", name="bqk_row")
            nc.sync.dma_start(bqk_row[:, :], d["bqk"][:, :])
            bv_row = persist.tile([1, CH], F16, tag="bv_row", name="bv_row")
            nc.sync.dma_start(bv_row[:, :], d["bv"][:, :])

        # ---- persistent f16 tensors (DMA'd pre-transposed from host) ----
        xT = [persist.tile([128, T], F16, tag=f"xT{j}", name=f"xT{j}") for j in range(NC_)]
        wqkT = [
            persist.tile([128, 2 * CH], F16, tag=f"wqkT{j}", name=f"wqkT{j}")
            for j in range(NC_)
        ]
        wvT = [
            persist.tile([128, CH], F16, tag=f"wvT{j}", name=f"wvT{j}")
            for j in range(NC_)
        ]
        wpT = [
            persist.tile([128, C], F16, tag=f"wpT{i}", name=f"wpT{i}") for i in range(2)
        ]
        qkT = [
            persist.tile([128, T], F16, tag=f"qkT{i}", name=f"qkT{i}") for i in range(4)
        ]
        Vt = [
            persist.tile([128, CH], F16, tag=f"Vt{t}", name=f"Vt{t}") for t in range(NT)
        ]
        attnT = [
            persist.tile([128, T], F16, tag=f"attnT{cb}", name=f"attnT{cb}")
            for cb in range(2)
        ]
        if use_lora_attn:
            AatT = [
                persist.tile([128, R], F16, tag=f"AatT{j}", name=f"AatT{j}")
                for j in range(NC_)
            ]
            BqkT = persist.tile([R, 2 * CH], F16, tag="BqkT", name="BqkT")
            BvT = persist.tile([R, CH], F16, tag="BvT", name="BvT")
            LT = persist.tile([R, T], F16, tag="LT", name="LT")
        if use_lora_proj:
            ApT = [
                persist.tile([128, R], F16, tag=f"ApT{i}", name=f"ApT{i}")
                for i in range(2)
            ]
            BpT = persist.tile([R, C], F16, tag="BpT", name="BpT")
            LpT = persist.tile([R, T], F16, tag="LpT", name="LpT")

        # ---- DMA weights + x^T (T-block-major so tb=0 unblocks early) ----
        for j in range(NC_):
            nc.sync.dma_start(xT[j][:, 0:512], d["xT"][j * 128 : (j + 1) * 128, 0:512])
            nc.sync.dma_start(wqkT[j][:, :], d["wqkT"][j * 128 : (j + 1) * 128, :])
        for j in range(NC_):
            nc.sync.dma_start(wvT[j][:, :], d["wvT"][j * 128 : (j + 1) * 128, :])
        for tbk in range(1, 4):
            for j in range(NC_):
                nc.sync.dma_start(
                    xT[j][:, tbk * 512 : (tbk + 1) * 512],
                    d["xT"][j * 128 : (j + 1) * 128, tbk * 512 : (tbk + 1) * 512],
                )
        for i in range(2):
            nc.sync.dma_start(wpT[i][:, :], d["wpT"][i * 128 : (i + 1) * 128, :])
        if use_lora_attn:
            for j in range(NC_):
                nc.sync.dma_start(AatT[j][:, :], d["aatT"][j * 128 : (j + 1) * 128, :])
            nc.sync.dma_start(BqkT[:, :], d["bqkT"][:, :])
            nc.sync.dma_start(BvT[:, :], d["bvT"][:, :])
        if use_lora_proj:
            for i in range(2):
                nc.sync.dma_start(ApT[i][:, :], d["apT"][i * 128 : (i + 1) * 128, :])
            nc.sync.dma_start(BpT[:, :], d["bpT"][:, :])

        def fq_chain(dst_slice, src_ps, w):
            """fake_quant sans scale (folded into weights):
            dst = clip(round(src + zp), 0, 255) - zp  (integer-valued)."""
            if zp_zero:
                t1 = fqp.tile([128, 512], F32, tag="fq", name="fq1")
                nc.vector.tensor_scalar(
                    t1[:, 0:w], src_ps, 0.0, QMAX, OP.max, OP.min
                )
                nc.vector.tensor_scalar(
                    dst_slice, t1[:, 0:w], MAGIC, MAGIC, OP.add, OP.subtract
                )
            else:
                t1 = fqp.tile([128, 512], F32, tag="fq", name="fq1")
                nc.vector.tensor_scalar(
                    t1[:, 0:w], src_ps, zp_ap, 0.0, OP.add, OP.max
                )
                t2 = fqp.tile([128, 512], F32, tag="fq", name="fq2")
                nc.vector.tensor_scalar(
                    t2[:, 0:w], t1[:, 0:w], QMAX, MAGIC, OP.min, OP.add
                )
                nc.vector.tensor_scalar(
                    dst_slice, t2[:, 0:w], MAGIC, zp_ap, OP.subtract, OP.subtract
                )

        # ================== PE work-unit emitters ==================
        def emit_lt(tb):  # lora attn intermediate for one T-block
            ps = psQ.tile([128, 512], F32, tag="q", name="lt_ps")[0:R, :]
            for j in range(NC_):
                nc.tensor.matmul(
                    ps[:, :],
                    AatT[j][:, :],
                    xT[j][:, tb * 512 : (tb + 1) * 512],
                    start=(j == 0),
                    stop=(j == NC_ - 1),
                )
            nc.vector.tensor_scalar(
                LT[:, tb * 512 : (tb + 1) * 512], ps[:, :], ALPHA_OVER_R, None, OP.mult
            )

        def emit_lp(qb):  # lora proj intermediate for one q-block
            ps = psQ.tile([128, 512], F32, tag="q", name="lp_ps")[0:R, :]
            for cb in range(2):
                nc.tensor.matmul(
                    ps[:, :],
                    ApT[cb][:, :],
                    attnT[cb][:, qb * 512 : (qb + 1) * 512],
                    start=(cb == 0),
                    stop=(cb == 1),
                )
            nc.vector.tensor_scalar(
                LpT[:, qb * 512 : (qb + 1) * 512], ps[:, :], ALPHA_OVER_R, None, OP.mult
            )

        def emit_qkt(tb, ct, half):
            """qkT chunk: half of the K=1024 accumulation for one (tb, ct).
            half=0 opens the psum, half=1 finishes + drains."""
            key = (tb, ct)
            if half == 0:
                qk_live[key] = psQ.tile([128, 512], F32, tag="q", name=f"qk{tb}{ct}")
            ps = qk_live[key]
            j0 = 0 if half == 0 else 4
            last = None if (use_lora_attn or use_bias) else NC_ - 1
            for j in range(j0, j0 + 4):
                nc.tensor.matmul(
                    ps[:, :],
                    wqkT[j][:, ct * 128 : (ct + 1) * 128],
                    xT[j][:, tb * 512 : (tb + 1) * 512],
                    start=(j == 0),
                    stop=(j == last),
                )
            if half == 1:
                if use_lora_attn:
                    nc.tensor.matmul(
                        ps[:, :],
                        BqkT[:, ct * 128 : (ct + 1) * 128],
                        LT[:, tb * 512 : (tb + 1) * 512],
                        start=False,
                        stop=(not use_bias),
                    )
                if use_bias:
                    nc.tensor.matmul(
                        ps[:, :],
                        bqk_row[:, ct * 128 : (ct + 1) * 128],
                        ones_row[:, 0:512],
                        start=False,
                        stop=True,
                    )
                dst = qkT[ct][:, tb * 512 : (tb + 1) * 512]
                if ct < 2:
                    nc.vector.tensor_copy(dst, ps[:, :])
                else:
                    fq_chain(dst, ps[:, :], 512)
                del qk_live[key]

        def emit_v(t):  # V for one T-tile (natural layout, integer-valued)
            ps = psQ.tile([128, 512], F32, tag="q", name=f"v{t}")[:, 0:CH]
            last = None if (use_lora_attn or use_bias) else NC_ - 1
            for j in range(NC_):
                nc.tensor.matmul(
                    ps[:, :],
                    xT[j][:, t * 128 : (t + 1) * 128],
                    wvT[j][:, :],
                    start=(j == 0),
                    stop=(j == last),
                )
            if use_lora_attn:
                nc.tensor.matmul(
                    ps[:, :],
                    LT[:, t * 128 : (t + 1) * 128],
                    BvT[:, :],
                    start=False,
                    stop=(not use_bias),
                )
            if use_bias:
                nc.tensor.matmul(
                    ps[:, :], ones_row[:, 0:128], bv_row[:, :], start=False, stop=True
                )
            fq_chain(Vt[t][:, :], ps[:, :], CH)

        def emit_proj(tt):  # output projection for one T-tile
            ps1 = psQ.tile([128, 512], F32, tag="q", name=f"pj{tt}a")
            ps2 = psQ.tile([128, 512], F32, tag="q", name=f"pj{tt}b")
            for cb in range(2):
                st = cb == 0
                sp = cb == 1 and not use_lora_proj
                lhs = attnT[cb][:, tt * 128 : (tt + 1) * 128]
                nc.tensor.matmul(ps1[:, :], lhs, wpT[cb][:, 0:512], start=st, stop=sp)
                nc.tensor.matmul(ps2[:, :], lhs, wpT[cb][:, 512:1024], start=st, stop=sp)
            if use_lora_proj:
                lhs = LpT[:, tt * 128 : (tt + 1) * 128]
                nc.tensor.matmul(ps1[:, :], lhs, BpT[:, 0:512], start=False, stop=True)
                nc.tensor.matmul(ps2[:, :], lhs, BpT[:, 512:1024], start=False, stop=True)
            po = outp.tile([128, C], F16, tag="po", name=f"po{tt}")
            nc.vector.tensor_copy(po[:, 0:512], ps1[:, :])
            nc.vector.tensor_copy(po[:, 512:1024], ps2[:, :])
            nc.sync.dma_start(d["out"][tt * 128 : (tt + 1) * 128, :], po[:, :])

        # ================== stage machinery ==================
        qk_live = {}
        filler = []  # list of (est_ns, closure), consumed FIFO
        fill_pos = [0]

        def emit_fill(budget_ns):
            spent = 0
            while spent < budget_ns and fill_pos[0] < len(filler):
                est, fn = filler[fill_pos[0]]
                fill_pos[0] += 1
                fn()
                spent += est

        def emit_fill_to(idx):
            # force-drain every filler before idx (correctness: scores(qb)
            # must sit behind the qkT/V chunks they depend on in the PE FIFO)
            while fill_pos[0] < idx:
                est, fn = filler[fill_pos[0]]
                fill_pos[0] += 1
                fn()

        def push(est, fn):
            filler.append((est, fn))

        def push_qkv_block(tb):
            if use_lora_attn:
                push(1900, lambda tb=tb: emit_lt(tb))
            for ct in (2, 3, 0, 1):  # k channels first: scores need them sooner
                for half in (0, 1):
                    push(900, lambda tb=tb, ct=ct, h=half: emit_qkt(tb, ct, h))
            for t in range(4 * tb, 4 * tb + 4):
                push(950, lambda t=t: emit_v(t))

        def push_proj_block(qb):
            if use_lora_proj:
                push(500, lambda qb=qb: emit_lp(qb))
            for tt in range(4 * qb, 4 * qb + 4):
                push(950, lambda tt=tt: emit_proj(tt))

        # scores + exp + mask for stage s=(qb,hp), tile j
        def emit_scores(qb, hp, j):
            qt = qkT[hp]
            kt = qkT[2 + hp]
            jl = j - 4 * qb
            lo = max(jl, 0) * 128
            ps = psS.tile([128, 1024], F32, tag="st", name="st_ps")
            nc.tensor.matmul(
                ps[:, lo:512],
                kt[0:64, j * 128 : (j + 1) * 128],
                qt[0:64, qb * 512 + lo : (qb + 1) * 512],
                start=True,
                stop=True,
                tile_position=(0, 0),
            )
            nc.tensor.matmul(
                ps[:, 512 + lo : 1024],
                kt[64:128, j * 128 : (j + 1) * 128],
                qt[64:128, qb * 512 + lo : (qb + 1) * 512],
                start=True,
                stop=True,
                tile_position=(64, 0),
            )
            ex = exq.tile([128, 1024], F16, tag="ex", name=f"ex{j}")
            exv = ex[:, :].rearrange("p (h q) -> p h q", q=512)[:, :, lo:512]
            psv = ps[:, :].rearrange("p (h q) -> p h q", q=512)[:, :, lo:512]
            nc.scalar.activation(exv, psv, AF.Exp, scale=es_ap)
            if jl >= 0:
                exd = ex[:, :].rearrange("p (h q) -> p h q", q=512)[
                    :, :, jl * 128 : jl * 128 + 128
                ]
                nc.gpsimd.tensor_tensor(
                    exd,
                    exd,
                    maskt[:, :]
                    .rearrange("p (o f) -> p o f", o=1)
                    .broadcast_to([128, 2, 128]),
                    OP.mult,
                )
            if DBG and qb == 0 and hp == 0 and j == 0:
                nc.sync.dma_start(d["dbg_ex00"][:, :], ex[:, :])
            return ex

        # PV + den contribution of tile j for stage s=(qb,hp)
        def emit_pvden(st, j):
            if st[4] is None:
                st[4] = psPV.tile([128, 512], F32, tag="pv", name=f"pv{st[0]}_{st[1]}")
                st[5] = psD.tile([128, 512], F32, tag="dn", name=f"dn{st[0]}_{st[1]}")
            qb, hp, nj, ex_tiles, pv, dn = st
            jl = j - 4 * qb
            lo = max(jl, 0) * 128
            ex = ex_tiles[j]
            first, last = j == 0, j == nj - 1
            for hh in range(2):
                nc.tensor.matmul(
                    pv[hh * 64 : (hh + 1) * 64, lo:512],
                    Vt[j][:, hp * 128 + hh * 64 : hp * 128 + (hh + 1) * 64],
                    ex[:, hh * 512 + lo : hh * 512 + 512],
                    start=first,
                    stop=last,
                    tile_position=(0, hh * 64),
                )
            nc.tensor.matmul(
                dn[0:1, lo:512],
                ones16[:, :],
                ex[:, lo:512],
                start=first,
                stop=last,
                skip_group_check=True,
            )

        def close_stage(st):
            qb, hp, nj, ex_tiles, pv, dn = st[:6]
            rcpA = rcpp.tile([1, 512], F32, tag="rcpr", name="rcpA")
            nc.vector.reciprocal(rcpA[0:1, :], dn[0:1, :])
            rcpA16 = rcpp.tile([1, 512], F16, tag="rcpr16", name="rcpA16")
            nc.vector.tensor_copy(rcpA16[0:1, :], rcpA[0:1, :])
            # den chain for the second head (psD slot freed by rcpA)
            dnB = psD.tile([128, 512], F32, tag="dn", name=f"dnB{qb}_{hp}")
            for j in range(nj):
                lo = max(j - 4 * qb, 0) * 128
                nc.tensor.matmul(
                    dnB[0:1, lo:512],
                    ones16[:, :],
                    ex_tiles[j][:, 512 + lo : 1024],
                    start=(j == 0),
                    stop=(j == nj - 1),
                    skip_group_check=True,
                )
            rcpB = rcpp.tile([1, 512], F32, tag="rcpr", name="rcpB")
            nc.vector.reciprocal(rcpB[0:1, :], dnB[0:1, :])
            rcpB16 = rcpp.tile([1, 512], F16, tag="rcpr16", name="rcpB16")
            nc.vector.tensor_copy(rcpB16[0:1, :], rcpB[0:1, :])
            # replicate the two reciprocal rows across partitions on the PE
            rb_ps = psD.tile([128, 512], F32, tag="dn", name=f"rb{qb}_{hp}")
            nc.tensor.matmul(rb_ps[:, :], mka[:, :], rcpA16[0:1, :],
                             start=True, stop=False, skip_group_check=True)
            nc.tensor.matmul(rb_ps[:, :], mkb[:, :], rcpB16[0:1, :],
                             start=False, stop=True, skip_group_check=True)
            rcpb = rcpp.tile([128, 512], F32, tag="rcpb", name="rcpb")
            nc.vector.tensor_copy(rcpb[:, :], rb_ps[:, :])
            nc.vector.tensor_tensor(
                attnT[hp][:, qb * 512 : (qb + 1) * 512], pv[:, :], rcpb[:, :], OP.mult
            )
            if DBG and qb == 0 and hp == 0:
                nc.sync.dma_start(d["dbg_rcp2"][:, :], rcp2[:, :])
                nc.sync.dma_start(d["dbg_rcpb"][:, :], rcpb[:, :])

        # ================== the pipelined stream ==================
        if SCHED == 0:
            # sequential reference order: no cross-stage pipelining
            for tb in range(4):
                push_qkv_block(tb)
            emit_fill(1 << 30)
            for s in range(8):
                qb, hp = s // 2, s % 2
                nj = 4 * qb + 4
                ex_tiles = []
                cur = [qb, hp, nj, ex_tiles, None, None]
                for j in range(nj):
                    ex_tiles.append(emit_scores(qb, hp, j))
                for j in range(nj):
                    emit_pvden(cur, j)
                close_stage(cur)
                if hp == 1:
                    push_proj_block(qb)
                    emit_fill(1 << 30)
            assert fill_pos[0] == len(filler)
            if DBG:
                nc.sync.dma_start(d["dbg_qkT0"][:, :], qkT[0][:, :])
                nc.sync.dma_start(d["dbg_qkT2"][:, :], qkT[2][:, :])
                nc.sync.dma_start(d["dbg_Vt0"][:, :], Vt[0][:, :])
                nc.sync.dma_start(d["dbg_attnT0"][:, :], attnT[0][:, :])
            return
        # prologue: qkv for T-block 0 (k channels first)
        qkv_end = {}
        push_qkv_block(0)
        qkv_end[0] = len(filler)
        emit_fill_to(qkv_end[0])
        push_qkv_block(1)
        qkv_end[1] = len(filler)

        prev = None
        for s in range(8):
            qb, hp = s // 2, s % 2
            nj = 4 * qb + 4
            if qb + 2 <= 3 and hp == 0:
                push_qkv_block(qb + 2)
                qkv_end[qb + 2] = len(filler)
            if hp == 0:
                # correctness: qkT/V chunks for this q-block must precede its
                # scores/PV in the PE FIFO
                emit_fill_to(qkv_end[qb])
            ex_tiles = []
            cur = [qb, hp, nj, ex_tiles, None, None]
            for j in range(nj):
                ex_tiles.append(emit_scores(qb, hp, j))
                if prev is not None and j < prev[2]:
                    emit_pvden(prev, j)
                    emit_fill(250)
                else:
                    emit_fill(1100)
            if prev is not None:
                close_stage(prev)
                if prev[1] == 1:  # finished both head pairs of q-block prev[0]
                    push_proj_block(prev[0])
            prev = cur
        # epilogue: drain last stage + remaining fillers
        for j in range(prev[2]):
            emit_pvden(prev, j)
            emit_fill(400)
        close_stage(prev)
        push_proj_block(prev[0])
        emit_fill(1 << 30)
        assert fill_pos[0] == len(filler)
        if DBG:
            nc.sync.dma_start(d["dbg_qkT0"][:, :], qkT[0][:, :])
            nc.sync.dma_start(d["dbg_qkT2"][:, :], qkT[2][:, :])
            nc.sync.dma_start(d["dbg_Vt0"][:, :], Vt[0][:, :])
            nc.sync.dma_start(d["dbg_attnT0"][:, :], attnT[0][:, :])


def _build_program(use_bias, use_lora_attn, use_lora_proj, zp_zero):
    nc = bacc.Bacc("TRN2", target_bir_lowering=False, debug=False, num_devices=N_CORES)

    def din(name, shape, dt=F16):
        return nc.dram_tensor(name, shape, dt, kind="ExternalInput").ap()

    d = {
        "xT": din("xT", [C, T]),
        "wqkT": din("wqkT", [C, 2 * CH]),
        "wvT": din("wvT", [C, CH]),
        "wpT": din("wpT", [CH, C]),
        "aatT": din("aatT", [C, R]),
        "bqkT": din("bqkT", [R, 2 * CH]),
        "bvT": din("bvT", [R, CH]),
        "apT": din("apT", [CH, R]),
        "bpT": din("bpT", [R, C]),
        "bqk": din("bqk", [1, 2 * CH]),
        "bv": din("bv", [1, CH]),
        "consts": din("consts", [128, 4], F32),
        "masks": din("masks", [128, 128]),
        "out": nc.dram_tensor("out", [T, C], F16, kind="ExternalOutput").ap(),
    }
    if DEBUG_DUMPS:
        d["dbg_qkT0"] = nc.dram_tensor("dbg_qkT0", [128, T], F16, kind="ExternalOutput").ap()
        d["dbg_qkT2"] = nc.dram_tensor("dbg_qkT2", [128, T], F16, kind="ExternalOutput").ap()
        d["dbg_Vt0"] = nc.dram_tensor("dbg_Vt0", [128, CH], F16, kind="ExternalOutput").ap()
        d["dbg_attnT0"] = nc.dram_tensor("dbg_attnT0", [128, T], F16, kind="ExternalOutput").ap()
        d["dbg_ex00"] = nc.dram_tensor("dbg_ex00", [128, 1024], F16, kind="ExternalOutput").ap()
        d["dbg_rcp2"] = nc.dram_tensor("dbg_rcp2", [65, 512], F32, kind="ExternalOutput").ap()
        d["dbg_rcpb"] = nc.dram_tensor("dbg_rcpb", [128, 512], F32, kind="ExternalOutput").ap()
    with tile.TileContext(nc) as tc:
        _build_body(nc, tc, d, use_bias, use_lora_attn, use_lora_proj, zp_zero)
    nc.compile()
    _dedupe_ldweights(nc)
    return nc


def _dedupe_ldweights(nc):
    """Remove back-to-back InstLdweights that reload identical weights."""
    removed = 0
    pe = mybir.EngineType.PE
    for blk in nc.m.functions[0].blocks:
        insts = blk.instructions
        keep = []
        prev_key = None
        for inst in insts:
            if getattr(inst, "engine", None) != pe:
                keep.append(inst)
                continue
            t = type(inst).__name__
            if t == "InstLdweights":
                si = inst.sync_info
                clean = si is None or (not si.on_wait and not si.on_update)
                key = str(inst.ins[0])
                if clean and prev_key is not None and key == prev_key:
                    removed += 1
                    continue
                prev_key = key
            elif t == "InstMatmult":
                if getattr(inst, "is_transpose", False):
                    prev_key = None
            keep.append(inst)
        if len(keep) != len(insts):
            blk.instructions = keep
    return removed


DEBUG_DUMPS = False
SCHED = 1
_CACHE = {}


def get_program(use_bias=True, use_lora_attn=True, use_lora_proj=True, zp_zero=False):
    key = (use_bias, use_lora_attn, use_lora_proj, zp_zero)
    if key not in _CACHE:
        _CACHE[key] = _build_program(*key)
    return _CACHE[key]


def make_in_maps(
    hidden_states, W_attn, b_attn, A_attn, B_attn, W_proj, b_proj, A_proj, B_proj,
    kv_scale, kv_zp,
):
    f32, f16 = np.float32, np.float16
    hidden_states = np.asarray(hidden_states, f32)
    W_attn = np.asarray(W_attn, f32)
    b_attn = np.asarray(b_attn, f32)
    A_attn = np.asarray(A_attn, f32)
    B_attn = np.asarray(B_attn, f32)
    W_proj = np.asarray(W_proj, f32)
    A_proj = np.asarray(A_proj, f32)
    B_proj = np.asarray(B_proj, f32)
    scale = f32(np.asarray(kv_scale, f32).reshape(-1)[0])
    zp = f32(np.asarray(kv_zp, f32).reshape(-1)[0])
    inv = f32(1.0) / scale

    consts = np.zeros((128, 4), f32)
    consts[:, 1] = zp
    consts[:, 3] = np.float32(0.125) * scale

    iota_p = np.arange(128)[:, None]
    iota_f = np.arange(128)[None, :]
    masks = (iota_f - iota_p >= 0).astype(f16)  # [128,128] upper-tri incl diag

    ct = lambda a: np.ascontiguousarray(a).astype(f16)
    xTs = [ct(hidden_states[b].T) for b in range(B)]
    bpT = ct(B_proj.T)

    in_maps = []
    for c in range(N_CORES):
        b = c // 4
        hg = c % 4
        qs = slice(hg * CH, (hg + 1) * CH)
        ks = slice(C + hg * CH, C + (hg + 1) * CH)
        vs = slice(2 * C + hg * CH, 2 * C + (hg + 1) * CH)
        wqk = np.concatenate([W_attn[qs], W_attn[ks] * inv], axis=0)
        bqkl = np.concatenate([B_attn[qs], B_attn[ks] * inv], axis=0)
        in_maps.append(
            {
                "xT": xTs[b],
                "wqkT": ct(wqk.T),
                "wvT": ct(W_attn[vs].T * inv),
                "wpT": ct(W_proj[:, hg * CH : (hg + 1) * CH].T * scale),
                "aatT": ct(A_attn.T),
                "bqkT": ct(bqkl.T),
                "bvT": ct(B_attn[vs].T * inv),
                "apT": ct(A_proj[:, hg * CH : (hg + 1) * CH].T * scale),
                "bpT": bpT,
                "bqk": ct(np.concatenate([b_attn[qs], b_attn[ks] * inv])[None, :]),
                "bv": ct(b_attn[vs][None, :] * inv),
                "consts": consts,
                "masks": masks,
            }
        )
    return in_maps


def variant_flags(b_attn, B_attn, B_proj, kv_zp=None):
    return (
        bool(np.any(np.asarray(b_attn))),
        bool(np.any(np.asarray(B_attn))),
        bool(np.any(np.asarray(B_proj))),
        not bool(np.any(np.asarray(kv_zp))) if kv_zp is not None else True,
    )


def assemble_output(results, b_proj):
    out = np.zeros((B, T, C), np.float32)
    for c in range(N_CORES):
        out[c // 4] += results[c]["out"].astype(np.float32)
    out += np.asarray(b_proj, np.float32)[None, None, :]
    return out


def kernel(**inputs):
    flags = variant_flags(
        inputs["b_attn"], inputs["B_attn"], inputs["B_proj"], inputs["kv_zp"]
    )
    nc = get_program(*flags)
    in_maps = make_in_maps(**inputs)
    res = run_bass_kernel_spmd(nc, in_maps, core_ids=list(range(N_CORES)))
    return assemble_output(res.results, inputs["b_proj"])


# revision 21
# speedup vs baseline: 1.0086x; 1.0086x over previous
"""CPT attention (QKV+LoRA -> fake-quant KV -> causal attention -> proj+LoRA)
as a Bass/Tile kernel on 8 TRN2 NeuronCores.

Sharding: data parallel over batch (2) x tensor parallel over heads (16/4=4
per core), Megatron-style. Each core computes qkv for its 4 heads from the
full hidden_states[b], runs causal attention locally, and produces a partial
projection output [T, C]; the host sums the 4 tensor-parallel partials per
batch and adds b_proj.

Device-side design (v2):
- All matmul operands f16, fp32 PSUM accum. Host pre-transposes everything.
- The quant scale 1/kv_scale is folded into the K/V columns of W_attn on the
  host, so fake_quant is 2 DVE passes (clip; magic-round) when kv_zp==0
  (3 passes otherwise). K is kept integer-valued (scale folded into the exp
  scale), V integer-valued (scale folded into W_proj).
- Scores are computed transposed (S^T[k, q]) per head pair with explicit PE
  row-group tile positions; exp on the scalar engine; causal diagonal masks
  multiplied on gpsimd.
- PV uses column-tiled matmul pairs: V_h0 -> PSUM partitions 0-63 and
  V_h1 -> partitions 64-127 stream two moving halves of ex concurrently,
  producing attnT[ch, q] directly (no transposes).
- Softmax denominators come from ones-stationary matmuls into a [2, 512]
  PSUM tile; reciprocal on DVE, replicated across partitions on gpsimd, and
  multiplied into the PV PSUM->SBUF evacuation.
- The whole kernel is one software-pipelined stream over 8 stages
  (4 q-blocks x 2 head pairs): scores(s) interleave with PV+den(s-1) and a
  filler queue of qkv/V/proj chunks sized to keep the PE busy while the
  scalar engine exps.
"""

import numpy as np

import concourse.bass as bass
import concourse.bacc as bacc
import concourse.mybir as mybir
import concourse.tile as tile
from concourse.bass_utils import run_bass_kernel_spmd

AF = mybir.ActivationFunctionType
OP = mybir.AluOpType

B, T, C = 2, 2048, 1024
H, HD = 16, 64
R = 16
ALPHA_OVER_R = 2.0
QMAX = 255.0
MAGIC = 12582912.0  # 1.5 * 2**23: fp32 add/sub rounds to nearest-even integer
N_CORES = 8
HPC = 4  # heads per core
CH = HPC * HD  # 256 v/q/k channels per core
NT = T // 128  # 16 T-tiles
NC_ = C // 128  # 8 C-tiles
F16 = mybir.dt.float16
F32 = mybir.dt.float32


def _build_body(nc, tc, d, use_bias, use_lora_attn, use_lora_proj, zp_zero):
    DBG = "dbg_qkT0" in d
    SCHED = globals()["SCHED"]
    import contextlib

    ctx = contextlib.ExitStack()
    with ctx:
        persist = ctx.enter_context(tc.tile_pool(name="persist", bufs=1))
        fqp = ctx.enter_context(tc.tile_pool(name="fqp", bufs=4))
        exq = ctx.enter_context(tc.tile_pool(name="exq", bufs=36))
        outp = ctx.enter_context(tc.tile_pool(name="outp", bufs=3))
        rcpp = ctx.enter_context(tc.tile_pool(name="rcpp", bufs=4))
        psS = ctx.enter_context(
            tc.tile_pool(name="psS", bufs=2, space=bass.MemorySpace.PSUM)
        )
        psQ = ctx.enter_context(
            tc.tile_pool(name="psQ", bufs=2, space=bass.MemorySpace.PSUM)
        )
        psPV = ctx.enter_context(
            tc.tile_pool(name="psPV", bufs=1, space=bass.MemorySpace.PSUM)
        )
        psD = ctx.enter_context(
            tc.tile_pool(name="psD", bufs=1, space=bass.MemorySpace.PSUM)
        )

        # ---- constants ----
        consts = persist.tile([128, 4], F32, tag="consts", name="consts")
        nc.sync.dma_start(consts[:, :], d["consts"][:, :])
        zp_ap = consts[:, 1:2]
        es_ap = consts[:, 3:4]  # 0.125 * kv_scale (scores use integer-valued K)
        maskt = persist.tile([128, 128], F16, tag="maskt", name="maskt")
        nc.sync.dma_start(maskt[:, :], d["masks"][:, :])
        ones16 = persist.tile([128, 1], F16, tag="ones16", name="ones16")
        nc.gpsimd.memset(ones16[:, :], 1.0)
        # rank-1 broadcast masks: mka selects out partitions 0-63, mkb 64-127
        mka = persist.tile([1, 128], F16, tag="mka", name="mka")
        nc.gpsimd.memset(mka[:, :], 0.0)
        nc.gpsimd.memset(mka[0:1, 0:64], 1.0)
        mkb = persist.tile([1, 128], F16, tag="mkb", name="mkb")
        nc.gpsimd.memset(mkb[:, :], 0.0)
        nc.gpsimd.memset(mkb[0:1, 64:128], 1.0)
        # preload the Exp activation table while DMAs stream in
        scr = persist.tile([1, 8], F32, tag="scr", name="scr")
        nc.scalar.activation(scr[0:1, 0:1], consts[0:1, 1:2], AF.Exp)
        if use_bias:
            ones_row = persist.tile([1, 512], F16, tag="ones_row", name="ones_row")
            nc.gpsimd.memset(ones_row[:, :], 1.0)
            bqk_row = persist.tile([1, 2 * CH], F16, tag="bqk_row", name="bqk_row")
            nc.sync.dma_start(bqk_row[:, :], d["bqk"][:, :])
            bv_row = persist.tile([1, CH], F16, tag="bv_row", name="bv_row")
            nc.sync.dma_start(bv_row[:, :], d["bv"][:, :])

        # ---- persistent f16 tensors (DMA'd pre-transposed from host) ----
        xT = [persist.tile([128, T], F16, tag=f"xT{j}", name=f"xT{j}") for j in range(NC_)]
        wqkT = [
            persist.tile([128, 2 * CH], F16, tag=f"wqkT{j}", name=f"wqkT{j}")
            for j in range(NC_)
        ]
        wvT = [
            persist.tile([128, CH], F16, tag=f"wvT{j}", name=f"wvT{j}")
            for j in range(NC_)
        ]
        wpT = [
            persist.tile([128, C], F16, tag=f"wpT{i}", name=f"wpT{i}") for i in range(2)
        ]
        qkT = [
            persist.tile([128, T], F16, tag=f"qkT{i}", name=f"qkT{i}") for i in range(4)
        ]
        Vt = [
            persist.tile([128, CH], F16, tag=f"Vt{t}", name=f"Vt{t}") for t in range(NT)
        ]
        attnT = [
            persist.tile([128, T], F16, tag=f"attnT{cb}", name=f"attnT{cb}")
            for cb in range(2)
        ]
        if use_lora_attn:
            AatT = [
                persist.tile([128, R], F16, tag=f"AatT{j}", name=f"AatT{j}")
                for j in range(NC_)
            ]
            BqkT = persist.tile([R, 2 * CH], F16, tag="BqkT", name="BqkT")
            BvT = persist.tile([R, CH], F16, tag="BvT", name="BvT")
            LT = persist.tile([R, T], F16, tag="LT", name="LT")
        if use_lora_proj:
            ApT = [
                persist.tile([128, R], F16, tag=f"ApT{i}", name=f"ApT{i}")
                for i in range(2)
            ]
            BpT = persist.tile([R, C], F16, tag="BpT", name="BpT")
            LpT = persist.tile([R, T], F16, tag="LpT", name="LpT")

        # ---- DMA weights + x^T (T-block-major so tb=0 unblocks early) ----
        for j in range(NC_):
            nc.sync.dma_start(xT[j][:, 0:512], d["xT"][j * 128 : (j + 1) * 128, 0:512])
            nc.sync.dma_start(wqkT[j][:, :], d["wqkT"][j * 128 : (j + 1) * 128, :])
        for j in range(NC_):
            nc.sync.dma_start(wvT[j][:, :], d["wvT"][j * 128 : (j + 1) * 128, :])
        for tbk in range(1, 4):
            for j in range(NC_):
                nc.sync.dma_start(
                    xT[j][:, tbk * 512 : (tbk + 1) * 512],
                    d["xT"][j * 128 : (j + 1) * 128, tbk * 512 : (tbk + 1) * 512],
                )
        for i in range(2):
            nc.sync.dma_start(wpT[i][:, :], d["wpT"][i * 128 : (i + 1) * 128, :])
        if use_lora_attn:
            for j in range(NC_):
                nc.sync.dma_start(AatT[j][:, :], d["aatT"][j * 128 : (j + 1) * 128, :])
            nc.sync.dma_start(BqkT[:, :], d["bqkT"][:, :])
            nc.sync.dma_start(BvT[:, :], d["bvT"][:, :])
        if use_lora_proj:
            for i in range(2):
                nc.sync.dma_start(ApT[i][:, :], d["apT"][i * 128 : (i + 1) * 128, :])
            nc.sync.dma_start(BpT[:, :], d["bpT"][:, :])

        def fq_chain(dst_slice, src_ps, w):
            """fake_quant sans scale (folded into weights):
            dst = clip(round(src + zp), 0, 255) - zp  (integer-valued)."""
            if zp_zero:
                t1 = fqp.tile([128, 512], F32, tag="fq", name="fq1")
                nc.vector.tensor_scalar(
                    t1[:, 0:w], src_ps, 0.0, QMAX, OP.max, OP.min
                )
                nc.vector.tensor_scalar(
                    dst_slice, t1[:, 0:w], MAGIC, MAGIC, OP.add, OP.subtract
                )
            else:
                t1 = fqp.tile([128, 512], F32, tag="fq", name="fq1")
                nc.vector.tensor_scalar(
                    t1[:, 0:w], src_ps, zp_ap, 0.0, OP.add, OP.max
                )
                t2 = fqp.tile([128, 512], F32, tag="fq", name="fq2")
                nc.vector.tensor_scalar(
                    t2[:, 0:w], t1[:, 0:w], QMAX, MAGIC, OP.min, OP.add
                )
                nc.vector.tensor_scalar(
                    dst_slice, t2[:, 0:w], MAGIC, zp_ap, OP.subtract, OP.subtract
                )

        # ================== PE work-unit emitters ==================
        def emit_lt(tb):  # lora attn intermediate for one T-block
            ps = psQ.tile([128, 512], F32, tag="q", name="lt_ps")[0:R, :]
            for j in range(NC_):
                nc.tensor.matmul(
                    ps[:, :],
                    AatT[j][:, :],
                    xT[j][:, tb * 512 : (tb + 1) * 512],
                    start=(j == 0),
                    stop=(j == NC_ - 1),
                )
            nc.vector.tensor_scalar(
                LT[:, tb * 512 : (tb + 1) * 512], ps[:, :], ALPHA_OVER_R, None, OP.mult
            )

        def emit_lp(qb):  # lora proj intermediate for one q-block
            ps = psQ.tile([128, 512], F32, tag="q", name="lp_ps")[0:R, :]
            for cb in range(2):
                nc.tensor.matmul(
                    ps[:, :],
                    ApT[cb][:, :],
                    attnT[cb][:, qb * 512 : (qb + 1) * 512],
                    start=(cb == 0),
                    stop=(cb == 1),
                )
            nc.vector.tensor_scalar(
                LpT[:, qb * 512 : (qb + 1) * 512], ps[:, :], ALPHA_OVER_R, None, OP.mult
            )

        def emit_qkt(tb, ct, half):
            """qkT chunk: half of the K=1024 accumulation for one (tb, ct).
            half=0 opens the psum, half=1 finishes + drains."""
            key = (tb, ct)
            if half == 0:
                qk_live[key] = psQ.tile([128, 512], F32, tag="q", name=f"qk{tb}{ct}")
            ps = qk_live[key]
            j0 = 0 if half == 0 else 4
            last = None if (use_lora_attn or use_bias) else NC_ - 1
            for j in range(j0, j0 + 4):
                nc.tensor.matmul(
                    ps[:, :],
                    wqkT[j][:, ct * 128 : (ct + 1) * 128],
                    xT[j][:, tb * 512 : (tb + 1) * 512],
                    start=(j == 0),
                    stop=(j == last),
                )
            if half == 1:
                if use_lora_attn:
                    nc.tensor.matmul(
                        ps[:, :],
                        BqkT[:, ct * 128 : (ct + 1) * 128],
                        LT[:, tb * 512 : (tb + 1) * 512],
                        start=False,
                        stop=(not use_bias),
                    )
                if use_bias:
                    nc.tensor.matmul(
                        ps[:, :],
                        bqk_row[:, ct * 128 : (ct + 1) * 128],
                        ones_row[:, 0:512],
                        start=False,
                        stop=True,
                    )
                dst = qkT[ct][:, tb * 512 : (tb + 1) * 512]
                if ct < 2:
                    nc.vector.tensor_copy(dst, ps[:, :])
                else:
                    fq_chain(dst, ps[:, :], 512)
                del qk_live[key]

        def emit_v(t):  # V for one T-tile (natural layout, integer-valued)
            ps = psQ.tile([128, 512], F32, tag="q", name=f"v{t}")[:, 0:CH]
            last = None if (use_lora_attn or use_bias) else NC_ - 1
            for j in range(NC_):
                nc.tensor.matmul(
                    ps[:, :],
                    xT[j][:, t * 128 : (t + 1) * 128],
                    wvT[j][:, :],
                    start=(j == 0),
                    stop=(j == last),
                )
            if use_lora_attn:
                nc.tensor.matmul(
                    ps[:, :],
                    LT[:, t * 128 : (t + 1) * 128],
                    BvT[:, :],
                    start=False,
                    stop=(not use_bias),
                )
            if use_bias:
                nc.tensor.matmul(
                    ps[:, :], ones_row[:, 0:128], bv_row[:, :], start=False, stop=True
                )
            fq_chain(Vt[t][:, :], ps[:, :], CH)

        def emit_proj(tt):  # output projection for one T-tile
            ps1 = psQ.tile([128, 512], F32, tag="q", name=f"pj{tt}a")
            ps2 = psQ.tile([128, 512], F32, tag="q", name=f"pj{tt}b")
            for cb in range(2):
                st = cb == 0
                sp = cb == 1 and not use_lora_proj
                lhs = attnT[cb][:, tt * 128 : (tt + 1) * 128]
                nc.tensor.matmul(ps1[:, :], lhs, wpT[cb][:, 0:512], start=st, stop=sp)
                nc.tensor.matmul(ps2[:, :], lhs, wpT[cb][:, 512:1024], start=st, stop=sp)
            if use_lora_proj:
                lhs = LpT[:, tt * 128 : (tt + 1) * 128]
                nc.tensor.matmul(ps1[:, :], lhs, BpT[:, 0:512], start=False, stop=True)
                nc.tensor.matmul(ps2[:, :], lhs, BpT[:, 512:1024], start=False, stop=True)
            po = outp.tile([128, C], F16, tag="po", name=f"po{tt}")
            nc.vector.tensor_copy(po[:, 0:512], ps1[:, :])
            nc.vector.tensor_copy(po[:, 512:1024], ps2[:, :])
            nc.sync.dma_start(d["out"][tt * 128 : (tt + 1) * 128, :], po[:, :])

        # ================== stage machinery ==================
        qk_live = {}
        filler = []  # list of (est_ns, closure), consumed FIFO
        fill_pos = [0]

        def emit_fill(budget_ns):
            spent = 0
            while spent < budget_ns and fill_pos[0] < len(filler):
                est, fn = filler[fill_pos[0]]
                fill_pos[0] += 1
                fn()
                spent += est

        def emit_fill_to(idx):
            # force-drain every filler before idx (correctness: scores(qb)
            # must sit behind the qkT/V chunks they depend on in the PE FIFO)
            while fill_pos[0] < idx:
                est, fn = filler[fill_pos[0]]
                fill_pos[0] += 1
                fn()

        def push(est, fn):
            filler.append((est, fn))

        def push_qkv_block(tb):
            if use_lora_attn:
                push(1900, lambda tb=tb: emit_lt(tb))
            for ct in (2, 3, 0, 1):  # k channels first: scores need them sooner
                for half in (0, 1):
                    push(900, lambda tb=tb, ct=ct, h=half: emit_qkt(tb, ct, h))
            for t in range(4 * tb, 4 * tb + 4):
                push(950, lambda t=t: emit_v(t))

        def push_proj_block(qb):
            if use_lora_proj:
                push(500, lambda qb=qb: emit_lp(qb))
            for tt in range(4 * qb, 4 * qb + 4):
                push(950, lambda tt=tt: emit_proj(tt))

        # scores + exp + mask for stage s=(qb,hp), tile j
        def emit_scores(qb, hp, j):
            qt = qkT[hp]
            kt = qkT[2 + hp]
            jl = j - 4 * qb
            lo = max(jl, 0) * 128
            ps = psS.tile([128, 1024], F32, tag="st", name="st_ps")
            nc.tensor.matmul(
                ps[:, lo:512],
                kt[0:64, j * 128 : (j + 1) * 128],
                qt[0:64, qb * 512 + lo : (qb + 1) * 512],
                start=True,
                stop=True,
                tile_position=(0, 0),
            )
            nc.tensor.matmul(
                ps[:, 512 + lo : 1024],
                kt[64:128, j * 128 : (j + 1) * 128],
                qt[64:128, qb * 512 + lo : (qb + 1) * 512],
                start=True,
                stop=True,
                tile_position=(64, 0),
            )
            ex = exq.tile([128, 1024], F16, tag="ex", name=f"ex{j}")
            exv = ex[:, :].rearrange("p (h q) -> p h q", q=512)[:, :, lo:512]
            psv = ps[:, :].rearrange("p (h q) -> p h q", q=512)[:, :, lo:512]
            nc.scalar.activation(exv, psv, AF.Exp, scale=es_ap)
            if jl >= 0:
                exd = ex[:, :].rearrange("p (h q) -> p h q", q=512)[
                    :, :, jl * 128 : jl * 128 + 128
                ]
                nc.vector.tensor_tensor(
                    exd,
                    exd,
                    maskt[:, :]
                    .rearrange("p (o f) -> p o f", o=1)
                    .broadcast_to([128, 2, 128]),
                    OP.mult,
                )
            if DBG and qb == 0 and hp == 0 and j == 0:
                nc.sync.dma_start(d["dbg_ex00"][:, :], ex[:, :])
            return ex

        # PV + den contribution of tile j for stage s=(qb,hp)
        def emit_pvden(st, j):
            if st[4] is None:
                st[4] = psPV.tile([128, 512], F32, tag="pv", name=f"pv{st[0]}_{st[1]}")
                st[5] = psD.tile([128, 512], F32, tag="dn", name=f"dn{st[0]}_{st[1]}")
            qb, hp, nj, ex_tiles, pv, dn = st
            jl = j - 4 * qb
            lo = max(jl, 0) * 128
            ex = ex_tiles[j]
            first, last = j == 0, j == nj - 1
            for hh in range(2):
                nc.tensor.matmul(
                    pv[hh * 64 : (hh + 1) * 64, lo:512],
                    Vt[j][:, hp * 128 + hh * 64 : hp * 128 + (hh + 1) * 64],
                    ex[:, hh * 512 + lo : hh * 512 + 512],
                    start=first,
                    stop=last,
                    tile_position=(0, hh * 64),
                )
            nc.tensor.matmul(
                dn[0:1, lo:512],
                ones16[:, :],
                ex[:, lo:512],
                start=first,
                stop=last,
                skip_group_check=True,
            )

        def close_stage(st):
            qb, hp, nj, ex_tiles, pv, dn = st[:6]
            with nc.allow_low_precision("softmax reciprocal rows in f16"):
                rcpA16 = rcpp.tile([1, 512], F16, tag="rcpr16", name="rcpA16")
                nc.vector.reciprocal(rcpA16[0:1, :], dn[0:1, :])
            # den chain for the second head (psD slot freed by rcpA16)
            dnB = psD.tile([128, 512], F32, tag="dn", name=f"dnB{qb}_{hp}")
            for j in range(nj):
                lo = max(j - 4 * qb, 0) * 128
                nc.tensor.matmul(
                    dnB[0:1, lo:512],
                    ones16[:, :],
                    ex_tiles[j][:, 512 + lo : 1024],
                    start=(j == 0),
                    stop=(j == nj - 1),
                    skip_group_check=True,
                )
            with nc.allow_low_precision("softmax reciprocal rows in f16"):
                rcpB16 = rcpp.tile([1, 512], F16, tag="rcpr16", name="rcpB16")
                nc.vector.reciprocal(rcpB16[0:1, :], dnB[0:1, :])
            # replicate the two reciprocal rows across partitions on the PE
            rb_ps = psD.tile([128, 512], F32, tag="dn", name=f"rb{qb}_{hp}")
            nc.tensor.matmul(rb_ps[:, :], mka[:, :], rcpA16[0:1, :],
                             start=True, stop=False, skip_group_check=True)
            nc.tensor.matmul(rb_ps[:, :], mkb[:, :], rcpB16[0:1, :],
                             start=False, stop=True, skip_group_check=True)
            rcpb = rcpp.tile([128, 512], F32, tag="rcpb", name="rcpb")
            nc.vector.tensor_copy(rcpb[:, :], rb_ps[:, :])
            nc.vector.tensor_tensor(
                attnT[hp][:, qb * 512 : (qb + 1) * 512], pv[:, :], rcpb[:, :], OP.mult
            )
            if DBG and qb == 0 and hp == 0:
                nc.sync.dma_start(d["dbg_rcp2"][:, :], rcp2[:, :])
                nc.sync.dma_start(d["dbg_rcpb"][:, :], rcpb[:, :])

        # ================== the pipelined stream ==================
        if SCHED == 0:
            # sequential reference order: no cross-stage pipelining
            for tb in range(4):
                push_qkv_block(tb)
            emit_fill(1 << 30)
            for s in range(8):
                qb, hp = s // 2, s % 2
                nj = 4 * qb + 4
                ex_tiles = []
                cur = [qb, hp, nj, ex_tiles, None, None]
                for j in range(nj):
                    ex_tiles.append(emit_scores(qb, hp, j))
                for j in range(nj):
                    emit_pvden(cur, j)
                close_stage(cur)
                if hp == 1:
                    push_proj_block(qb)
                    emit_fill(1 << 30)
            assert fill_pos[0] == len(filler)
            if DBG:
                nc.sync.dma_start(d["dbg_qkT0"][:, :], qkT[0][:, :])
                nc.sync.dma_start(d["dbg_qkT2"][:, :], qkT[2][:, :])
                nc.sync.dma_start(d["dbg_Vt0"][:, :], Vt[0][:, :])
                nc.sync.dma_start(d["dbg_attnT0"][:, :], attnT[0][:, :])
            return
        # prologue: qkv for T-block 0 (k channels first)
        qkv_end = {}
        push_qkv_block(0)
        qkv_end[0] = len(filler)
        emit_fill_to(qkv_end[0])
        push_qkv_block(1)
        qkv_end[1] = len(filler)

        prev = None
        prev2 = None
        for s in range(8):
            qb, hp = s // 2, s % 2
            nj = 4 * qb + 4
            if qb + 2 <= 3 and hp == 0:
                push_qkv_block(qb + 2)
                qkv_end[qb + 2] = len(filler)
            if hp == 0:
                # correctness: qkT/V chunks for this q-block must precede its
                # scores/PV in the PE FIFO
                emit_fill_to(qkv_end[qb])
            if prev2 is not None:
                # den_h0(prev2) stopped during the previous stage; the whole
                # close chain is ready now -> no engine-FIFO head blocking
                close_stage(prev2)
                if prev2[1] == 1:
                    push_proj_block(prev2[0])
            ex_tiles = []
            cur = [qb, hp, nj, ex_tiles, None, None]
            for j in range(nj):
                ex_tiles.append(emit_scores(qb, hp, j))
                if prev is not None and j < prev[2]:
                    emit_pvden(prev, j)
                    emit_fill(250)
                else:
                    emit_fill(1100)
            prev2 = prev
            prev = cur
        # epilogue: drain the last two stages + remaining fillers
        close_stage(prev2)
        push_proj_block(prev2[0])
        for j in range(prev[2]):
            emit_pvden(prev, j)
            emit_fill(400)
        close_stage(prev)
        push_proj_block(prev[0])
        emit_fill(1 << 30)
        assert fill_pos[0] == len(filler)
        if DBG:
            nc.sync.dma_start(d["dbg_qkT0"][:, :], qkT[0][:, :])
            nc.sync.dma_start(d["dbg_qkT2"][:, :], qkT[2][:, :])
            nc.sync.dma_start(d["dbg_Vt0"][:, :], Vt[0][:, :])
            nc.sync.dma_start(d["dbg_attnT0"][:, :], attnT[0][:, :])


def _build_program(use_bias, use_lora_attn, use_lora_proj, zp_zero):
    nc = bacc.Bacc("TRN2", target_bir_lowering=False, debug=False, num_devices=N_CORES)

    def din(name, shape, dt=F16):
        return nc.dram_tensor(name, shape, dt, kind="ExternalInput").ap()

    d = {
        "xT": din("xT", [C, T]),
        "wqkT": din("wqkT", [C, 2 * CH]),
        "wvT": din("wvT", [C, CH]),
        "wpT": din("wpT", [CH, C]),
        "aatT": din("aatT", [C, R]),
        "bqkT": din("bqkT", [R, 2 * CH]),
        "bvT": din("bvT", [R, CH]),
        "apT": din("apT", [CH, R]),
        "bpT": din("bpT", [R, C]),
        "bqk": din("bqk", [1, 2 * CH]),
        "bv": din("bv", [1, CH]),
        "consts": din("consts", [128, 4], F32),
        "masks": din("masks", [128, 128]),
        "out": nc.dram_tensor("out", [T, C], F16, kind="ExternalOutput").ap(),
    }
    if DEBUG_DUMPS:
        d["dbg_qkT0"] = nc.dram_tensor("dbg_qkT0", [128, T], F16, kind="ExternalOutput").ap()
        d["dbg_qkT2"] = nc.dram_tensor("dbg_qkT2", [128, T], F16, kind="ExternalOutput").ap()
        d["dbg_Vt0"] = nc.dram_tensor("dbg_Vt0", [128, CH], F16, kind="ExternalOutput").ap()
        d["dbg_attnT0"] = nc.dram_tensor("dbg_attnT0", [128, T], F16, kind="ExternalOutput").ap()
        d["dbg_ex00"] = nc.dram_tensor("dbg_ex00", [128, 1024], F16, kind="ExternalOutput").ap()
        d["dbg_rcp2"] = nc.dram_tensor("dbg_rcp2", [65, 512], F32, kind="ExternalOutput").ap()
        d["dbg_rcpb"] = nc.dram_tensor("dbg_rcpb", [128, 512], F32, kind="ExternalOutput").ap()
    with tile.TileContext(nc) as tc:
        _build_body(nc, tc, d, use_bias, use_lora_attn, use_lora_proj, zp_zero)
    nc.compile()
    _dedupe_ldweights(nc)
    return nc


def _dedupe_ldweights(nc):
    """Remove back-to-back InstLdweights that reload identical weights."""
    removed = 0
    pe = mybir.EngineType.PE
    for blk in nc.m.functions[0].blocks:
        insts = blk.instructions
        keep = []
        prev_key = None
        for inst in insts:
            if getattr(inst, "engine", None) != pe:
                keep.append(inst)
                continue
            t = type(inst).__name__
            if t == "InstLdweights":
                si = inst.sync_info
                clean = si is None or (not si.on_wait and not si.on_update)
                key = str(inst.ins[0])
                if clean and prev_key is not None and key == prev_key:
                    removed += 1
                    continue
                prev_key = key
            elif t == "InstMatmult":
                if getattr(inst, "is_transpose", False):
                    prev_key = None
            keep.append(inst)
        if len(keep) != len(insts):
            blk.instructions = keep
    return removed


DEBUG_DUMPS = False
SCHED = 1
_CACHE = {}


def get_program(use_bias=True, use_lora_attn=True, use_lora_proj=True, zp_zero=False):
    key = (use_bias, use_lora_attn, use_lora_proj, zp_zero)
    if key not in _CACHE:
        _CACHE[key] = _build_program(*key)
    return _CACHE[key]


def make_in_maps(
    hidden_states, W_attn, b_attn, A_attn, B_attn, W_proj, b_proj, A_proj, B_proj,
    kv_scale, kv_zp,
):
    f32, f16 = np.float32, np.float16
    hidden_states = np.asarray(hidden_states, f32)
    W_attn = np.asarray(W_attn, f32)
    b_attn = np.asarray(b_attn, f32)
    A_attn = np.asarray(A_attn, f32)
    B_attn = np.asarray(B_attn, f32)
    W_proj = np.asarray(W_proj, f32)
    A_proj = np.asarray(A_proj, f32)
    B_proj = np.asarray(B_proj, f32)
    scale = f32(np.asarray(kv_scale, f32).reshape(-1)[0])
    zp = f32(np.asarray(kv_zp, f32).reshape(-1)[0])
    inv = f32(1.0) / scale

    consts = np.zeros((128, 4), f32)
    consts[:, 1] = zp
    consts[:, 3] = np.float32(0.125) * scale

    iota_p = np.arange(128)[:, None]
    iota_f = np.arange(128)[None, :]
    masks = (iota_f - iota_p >= 0).astype(f16)  # [128,128] upper-tri incl diag

    ct = lambda a: np.ascontiguousarray(a).astype(f16)
    xTs = [ct(hidden_states[b].T) for b in range(B)]
    bpT = ct(B_proj.T)

    in_maps = []
    for c in range(N_CORES):
        b = c // 4
        hg = c % 4
        qs = slice(hg * CH, (hg + 1) * CH)
        ks = slice(C + hg * CH, C + (hg + 1) * CH)
        vs = slice(2 * C + hg * CH, 2 * C + (hg + 1) * CH)
        wqk = np.concatenate([W_attn[qs], W_attn[ks] * inv], axis=0)
        bqkl = np.concatenate([B_attn[qs], B_attn[ks] * inv], axis=0)
        in_maps.append(
            {
                "xT": xTs[b],
                "wqkT": ct(wqk.T),
                "wvT": ct(W_attn[vs].T * inv),
                "wpT": ct(W_proj[:, hg * CH : (hg + 1) * CH].T * scale),
                "aatT": ct(A_attn.T),
                "bqkT": ct(bqkl.T),
                "bvT": ct(B_attn[vs].T * inv),
                "apT": ct(A_proj[:, hg * CH : (hg + 1) * CH].T * scale),
                "bpT": bpT,
                "bqk": ct(np.concatenate([b_attn[qs], b_attn[ks] * inv])[None, :]),
                "bv": ct(b_attn[vs][None, :] * inv),
                "consts": consts,
                "masks": masks,
            }
        )
    return in_maps


def variant_flags(b_attn, B_attn, B_proj, kv_zp=None):
    return (
        bool(np.any(np.asarray(b_attn))),
        bool(np.any(np.asarray(B_attn))),
        bool(np.any(np.asarray(B_proj))),
        not bool(np.any(np.asarray(kv_zp))) if kv_zp is not None else True,
    )


def assemble_output(results, b_proj):
    out = np.zeros((B, T, C), np.float32)
    for c in range(N_CORES):
        out[c // 4] += results[c]["out"].astype(np.float32)
    out += np.asarray(b_proj, np.float32)[None, None, :]
    return out


def kernel(**inputs):
    flags = variant_flags(
        inputs["b_attn"], inputs["B_attn"], inputs["B_proj"], inputs["kv_zp"]
    )
    nc = get_program(*flags)
    in_maps = make_in_maps(**inputs)
    res = run_bass_kernel_spmd(nc, in_maps, core_ids=list(range(N_CORES)))
    return assemble_output(res.results, inputs["b_proj"])
